# revision 41
# baseline (speedup 1.0000x reference)
"""GPR surrogate prediction kernel for Trainium2 (8 NeuronCores, Bass/Tile).

Computes pred = K_star @ alpha where K_star = exp(-||x_m - xtrain_n||^2 / 2).

Math: exp(-(sq1[m] + sq2[n] - 2 x.y)/2) * alpha[n]
    = exp(x.y - sq1[m]/2) * (alpha[n] * exp(-sq2[n]/2))
so per core (M sharded 8 ways, layout [m=128 partitions, n free]):
  - TensorE:  dot[m, n] = X_c @ X_train.T            (bf16, fp32 PSUM)
  - ScalarE:  K[m, n] = exp(dot + bias[m]),  bias[m] = -sq1[m]/2  (per-partition bias)
  - VectorE:  pred[m] += sum_n K[m, n] * ar[n],  ar[n] = alpha[n]*exp(-sq2[n]/2)
              (scalar_tensor_tensor with dense bf16 out so the DVE 2x perf
              mode engages; accum_out gives the per-chunk column sum.)
The per-element K*ar product underflows fp32 exactly where the reference's
single exp underflows, so results match the reference bit-for-bit in the
all-underflow regime and to normal rounding otherwise.

Pipeline structure: n-groups outer, m-tiles inner, so compute starts after
the first 1 MB weight chunk lands and later group DMAs overlap compute.
"""

import functools

import ml_dtypes
import numpy as np

M, N, D = 4096, 8192, 256
NCORES = 8
P = 128
MC = M // NCORES          # 512 query rows per core
MT = MC // P              # 4 m-tiles per core
NTILE = 512               # matmul free dim (one PSUM bank)
NGRP = 2048               # n per PSUM group (4 banks) = one ACT batch
NGROUPS = N // NGRP       # 4
TPG = NGRP // NTILE       # 4 n-tiles per group
DCH = D // P              # 2 contraction chunks
ACCUM_G = 1               # sign-packed n-group whose reduce runs on ScalarE

BF16 = ml_dtypes.bfloat16


@functools.lru_cache(maxsize=1)
def _build():
    import concourse.bacc as bacc
    import concourse.mybir as mybir
    import concourse.tile as tile

    fp32 = mybir.dt.float32
    bf16 = mybir.dt.bfloat16

    nc = bacc.Bacc(
        "TRN2",
        target_bir_lowering=False,
        debug=False,
        enable_asserts=False,
        num_devices=NCORES,
    )

    xt = nc.dram_tensor("xt", [P, DCH, MC], bf16, kind="ExternalInput").ap()
    wt = nc.dram_tensor("wt", [P, DCH, N], bf16, kind="ExternalInput").ap()
    ar = nc.dram_tensor("ar", [P, N], bf16, kind="ExternalInput").ap()
    bias = nc.dram_tensor("bias", [P, MT], fp32, kind="ExternalInput").ap()
    # ln(ar) for the ACCUM_G columns as a bf16 hi/lo pair: row 0 + row 1
    # reconstruct ln(ar) to ~1e-3 absolute inside the fp32 PSUM accumulate.
    lnar = nc.dram_tensor("lnar", [2, NGRP], bf16, kind="ExternalInput").ap()
    y = nc.dram_tensor("y", [P, MT], fp32, kind="ExternalOutput").ap()

    with tile.TileContext(nc) as tc:
        with (
            tc.tile_pool(name="const", bufs=1) as cpool,
            tc.tile_pool(name="kpool", bufs=4) as kpool,
            tc.tile_pool(name="prod", bufs=4) as prpool,
            tc.tile_pool(name="scr", bufs=2) as spool,
            tc.tile_pool(name="psum", bufs=2, space="PSUM") as ppool,
        ):
            # Resident tensors
            xt_sb = cpool.tile([P, DCH, MC], bf16, name="xt_sb")
            wt_sb = cpool.tile([P, DCH, N], bf16, name="wt_sb")
            ar_sb = cpool.tile([P, N], bf16, name="ar_sb")
            bias_sb = cpool.tile([P, MT], fp32, name="bias_sb")
            # Chunk (g0, mt0) is split into two 1024-wide sub-chunks so the
            # pipeline starts as soon as the first half of the g0 weights
            # lands; mt0 therefore owns NGROUPS+1 partial-sum columns.
            acc_sb = cpool.tile([P, MT * NGROUPS + 1], fp32, name="acc_sb")
            ones_sb = cpool.tile([P, NGROUPS + 1], fp32, name="ones_sb")
            y_sb = cpool.tile([P, MT], fp32, name="y_sb")
            warm_sb = cpool.tile([P, 1], fp32, name="warm_sb")
            lnar_sb = cpool.tile([2, NGRP], bf16, name="lnar_sb")
            ones2_sb = cpool.tile([2, P], bf16, name="ones2_sb")
            dump_sb = cpool.tile([P, NGRP], bf16, name="dump_sb")

            # DMA triggers cost ~0.65us each on the Sync engine, so order
            # them by need: xt (first LDWEIGHTS), wt g0 d0 (first matmul),
            # then the rest. Later groups overlap compute.
            def load_group(g):
                gs = slice(g * NGRP, (g + 1) * NGRP)
                nc.sync.dma_start(wt_sb[:, :, gs], wt[:, :, gs])
                if g != ACCUM_G:
                    # The accum group folds ar into the matmul; no ar needed.
                    nc.sync.dma_start(ar_sb[:, gs], ar[:, gs])

            # DMA triggers serialize at ~0.65us each on the Sync engine, so
            # the four transfers the first 1024-wide sub-chunk needs lead
            # the queue; everything else streams behind them.
            HLF = NGRP // 2
            nc.sync.dma_start(xt_sb[:], xt[:])
            nc.sync.dma_start(wt_sb[:, :, 0:HLF], wt[:, :, 0:HLF])
            nc.sync.dma_start(bias_sb[:], bias[:])
            nc.sync.dma_start(ar_sb[:, 0:HLF], ar[:, 0:HLF])
            nc.sync.dma_start(wt_sb[:, :, HLF:NGRP], wt[:, :, HLF:NGRP])
            nc.sync.dma_start(ar_sb[:, HLF:NGRP], ar[:, HLF:NGRP])
            nc.sync.dma_start(lnar_sb[:], lnar[:])

            nc.vector.memset(ones_sb[:], 1.0)
            nc.vector.memset(ones2_sb[:], 1.0)
            # Dummy activation: hoists the ~2.7us exp table load into the
            # DMA head instead of stalling the first real chunk.
            nc.scalar.activation(
                warm_sb[:],
                ones_sb[:, 0:1],
                mybir.ActivationFunctionType.Exp,
                scale=1.0,
            )
            def do_chunk(g, mt, ci, n0, width, ps, po):
                """MMs + exp + weighted reduce for cols [n0, n0+width)."""
                ms = slice(mt * P, (mt + 1) * P)
                accum = g == ACCUM_G
                for d in range(DCH):
                    for t in range(width // NTILE):
                        o = t * NTILE
                        nc.tensor.matmul(
                            ps[:, po + o : po + o + NTILE],
                            lhsT=xt_sb[:, d, ms],
                            rhs=wt_sb[:, d, n0 + o : n0 + o + NTILE],
                            start=(d == 0),
                            stop=(d == DCH - 1 and not accum),
                        )
                if accum:
                    # Rank-2 update adds ln(ar) (hi+lo rows) to every dot;
                    # ScalarE's accumulator then does the weighted reduce,
                    # freeing the DVE for the other groups.
                    for t in range(width // NTILE):
                        o = t * NTILE
                        nc.tensor.matmul(
                            ps[:, po + o : po + o + NTILE],
                            lhsT=ones2_sb[:],
                            rhs=lnar_sb[:, n0 - ACCUM_G * NGRP + o :
                                        n0 - ACCUM_G * NGRP + o + NTILE],
                            start=False,
                            stop=True,
                        )
                    nc.scalar.activation(
                        dump_sb[:, 0:width],
                        ps[:, po : po + width],
                        mybir.ActivationFunctionType.Exp,
                        bias=bias_sb[:, mt : mt + 1],
                        scale=1.0,
                        accum_out=acc_sb[:, ci : ci + 1],
                    )
                    return
                k = kpool.tile([P, width], bf16, name="k")
                nc.scalar.activation(
                    k[:],
                    ps[:, po : po + width],
                    mybir.ActivationFunctionType.Exp,
                    bias=bias_sb[:, mt : mt + 1],
                    scale=1.0,
                )
                prod = prpool.tile([P, width], bf16, name="prod")
                nc.vector.scalar_tensor_tensor(
                    prod[:],
                    k[:],
                    1.0,
                    ar_sb[:, n0 : n0 + width],
                    op0=mybir.AluOpType.mult,
                    op1=mybir.AluOpType.mult,
                    accum_out=acc_sb[:, ci : ci + 1],
                )

            # mt0 partials live in columns 0..NGROUPS (one extra for the
            # split first chunk); mt>0 in NGROUPS+1+(mt-1)*NGROUPS+g.
            def col(mt, g):
                if mt == 0:
                    return g + 1  # g0 sub-chunks use columns 0 and 1
                return NGROUPS + 1 + (mt - 1) * NGROUPS + g

            for g in range(NGROUPS):
                if g + 1 < NGROUPS:
                    load_group(g + 1)
                for mt in range(MT):
                    ps = ppool.tile([P, NGRP], fp32, name="ps")
                    if g == 0 and mt == 0:
                        do_chunk(0, 0, 0, 0, HLF, ps, 0)
                        do_chunk(0, 0, 1, HLF, HLF, ps, HLF)
                    else:
                        do_chunk(g, mt, col(mt, g), g * NGRP, NGRP, ps, 0)
            for mt in range(MT):
                # Reduce this m-tile's partial-sum columns into one column.
                w = NGROUPS + 1 if mt == 0 else NGROUPS
                c0 = 0 if mt == 0 else NGROUPS + 1 + (mt - 1) * NGROUPS
                scrf = spool.tile([P, 1], fp32, name="scrf")
                nc.vector.scalar_tensor_tensor(
                    scrf.broadcast_to((P, w)),
                    acc_sb[:, c0 : c0 + w],
                    1.0,
                    ones_sb[:, 0:w],
                    op0=mybir.AluOpType.mult,
                    op1=mybir.AluOpType.mult,
                    accum_out=y_sb[:, mt : mt + 1],
                )
            nc.sync.dma_start(y[:], y_sb[:])

    nc.compile()
    return nc


def _prep_inputs(X, X_train, alpha):
    """Host-side layout prep: transposes, casts, norm terms. O((M+N)*D) work."""
    X = np.asarray(X, dtype=np.float32)
    X_train = np.asarray(X_train, dtype=np.float32)
    alpha = np.asarray(alpha, dtype=np.float32).reshape(-1)

    sq1 = np.sum(X.astype(np.float64) ** 2, axis=1)        # [M]
    sq2 = np.sum(X_train.astype(np.float64) ** 2, axis=1)  # [N]

    # alpha' = alpha * exp(-||xtrain||^2/2); fp64 -> fp32 cast underflows to 0
    # exactly where the reference's fp32 exp does (up to the bf16 flush, which
    # only drops terms < 1e-40).
    ar_row = (alpha.astype(np.float64) * np.exp(-sq2 / 2.0)).astype(np.float32)

    # Pack NGRP nonnegative-ar columns into group ACCUM_G: that group's
    # reduce runs as exp(dot + ln(ar) - sq1/2) summed on ScalarE, which
    # needs ln(ar) real, i.e. a uniformly nonnegative block.
    nonneg = np.flatnonzero(ar_row >= 0.0)
    assert len(nonneg) >= NGRP, "not enough nonnegative-alpha columns to pack"
    g_idx = nonneg[:NGRP]
    rest = np.setdiff1d(np.arange(N), g_idx)
    perm = np.concatenate(
        [rest[: ACCUM_G * NGRP], g_idx, rest[ACCUM_G * NGRP :]]
    )
    X_train = X_train[perm]
    ar_row = ar_row[perm]
    acc_cols = ar_row[ACCUM_G * NGRP : (ACCUM_G + 1) * NGRP].astype(np.float64)
    with np.errstate(divide="ignore"):
        lnar_row = np.where(acc_cols > 0, np.log(acc_cols), -1e30)
    hi = lnar_row.astype(BF16)
    lo = (lnar_row - hi.astype(np.float64)).astype(BF16)
    lnar_np = np.ascontiguousarray(np.stack([hi, lo]))  # [2, NGRP] bf16

    ar_np = np.ascontiguousarray(
        np.broadcast_to(ar_row.astype(BF16)[None, :], (P, N))
    )

    # [P, DCH, M]: partition-major layout so each tensor loads in one DMA.
    xt_full = np.ascontiguousarray(
        X.T.astype(BF16).reshape(DCH, P, M).transpose(1, 0, 2)
    )
    wt_np = np.ascontiguousarray(
        X_train.T.astype(BF16).reshape(DCH, P, N).transpose(1, 0, 2)
    )

    bias_full = (-sq1 / 2.0).astype(np.float32)  # [M]

    in_maps = []
    for c in range(NCORES):
        mslice = slice(c * MC, (c + 1) * MC)
        xt_c = np.ascontiguousarray(xt_full[:, :, mslice])  # [P, DCH, MC]
        # bias[p, mt] = -sq1[c*MC + mt*P + p]/2
        bias_c = np.ascontiguousarray(bias_full[mslice].reshape(MT, P).T)
        in_maps.append(
            {"xt": xt_c, "wt": wt_np, "ar": ar_np, "bias": bias_c, "lnar": lnar_np}
        )
    return in_maps


LAST_RES = None


def kernel(X, X_train, alpha):
    from concourse import bass_utils

    nc = _build()
    in_maps = _prep_inputs(X, X_train, alpha)
    res = bass_utils.run_bass_kernel_spmd(
        nc, in_maps, core_ids=list(range(NCORES))
    ).results
    global LAST_RES
    LAST_RES = res

    out = np.empty((M, 1), dtype=np.float32)
    for c in range(NCORES):
        yc = res[c]["y"]  # [P, MT]; column mt holds rows c*MC + mt*P .. +P
        out[c * MC : (c + 1) * MC, 0] = yc.T.reshape(MC)
    return out


if __name__ == "__main__":
    rng = np.random.default_rng(0)
    X = rng.standard_normal((M, D), dtype=np.float32)
    Xt = rng.standard_normal((N, D), dtype=np.float32)
    a = rng.standard_normal((N, 1), dtype=np.float32)
    out = kernel(X=X, X_train=Xt, alpha=a)
    print("out", out.shape, out.dtype, "nonzero:", np.count_nonzero(out))


# revision 42
# speedup vs baseline: 1.0515x; 1.0515x over previous
"""GPR surrogate prediction kernel for Trainium2 (8 NeuronCores, Bass/Tile).

Computes pred = K_star @ alpha where K_star = exp(-||x_m - xtrain_n||^2 / 2).

Math: exp(-(sq1[m] + sq2[n] - 2 x.y)/2) * alpha[n]
    = exp(x.y - sq1[m]/2) * (alpha[n] * exp(-sq2[n]/2))
so per core (M sharded 8 ways, layout [m=128 partitions, n free]):
  - TensorE:  dot[m, n] = X_c @ X_train.T            (bf16, fp32 PSUM)
  - ScalarE:  K[m, n] = exp(dot + bias[m]),  bias[m] = -sq1[m]/2  (per-partition bias)
  - VectorE:  pred[m] += sum_n K[m, n] * ar[n],  ar[n] = alpha[n]*exp(-sq2[n]/2)
              (scalar_tensor_tensor with dense bf16 out so the DVE 2x perf
              mode engages; accum_out gives the per-chunk column sum.)
The per-element K*ar product underflows fp32 exactly where the reference's
single exp underflows, so results match the reference bit-for-bit in the
all-underflow regime and to normal rounding otherwise.

Pipeline structure: n-groups outer, m-tiles inner, so compute starts after
the first 1 MB weight chunk lands and later group DMAs overlap compute.
"""

import functools

import ml_dtypes
import numpy as np

M, N, D = 4096, 8192, 256
NCORES = 8
P = 128
MC = M // NCORES          # 512 query rows per core
MT = MC // P              # 4 m-tiles per core
NTILE = 512               # matmul free dim (one PSUM bank)
NGRP = 2048               # n per PSUM group (4 banks) = one ACT batch
NGROUPS = N // NGRP       # 4
TPG = NGRP // NTILE       # 4 n-tiles per group
DCH = D // P              # 2 contraction chunks

BF16 = ml_dtypes.bfloat16


@functools.lru_cache(maxsize=1)
def _build():
    import concourse.bacc as bacc
    import concourse.mybir as mybir
    import concourse.tile as tile

    fp32 = mybir.dt.float32
    bf16 = mybir.dt.bfloat16

    nc = bacc.Bacc(
        "TRN2",
        target_bir_lowering=False,
        debug=False,
        enable_asserts=False,
        num_devices=NCORES,
    )

    xt = nc.dram_tensor("xt", [P, DCH, MC], bf16, kind="ExternalInput").ap()
    wt = nc.dram_tensor("wt", [P, DCH, N], bf16, kind="ExternalInput").ap()
    ar = nc.dram_tensor("ar", [P, N], bf16, kind="ExternalInput").ap()
    bias = nc.dram_tensor("bias", [P, MT], fp32, kind="ExternalInput").ap()
    y = nc.dram_tensor("y", [P, MT], fp32, kind="ExternalOutput").ap()

    with tile.TileContext(nc) as tc:
        with (
            tc.tile_pool(name="const", bufs=1) as cpool,
            tc.tile_pool(name="kpool", bufs=4) as kpool,
            tc.tile_pool(name="prod", bufs=4) as prpool,
            tc.tile_pool(name="scr", bufs=2) as spool,
            tc.tile_pool(name="psum", bufs=2, space="PSUM") as ppool,
        ):
            # Resident tensors
            xt_sb = cpool.tile([P, DCH, MC], bf16, name="xt_sb")
            wt_sb = cpool.tile([P, DCH, N], bf16, name="wt_sb")
            ar_sb = cpool.tile([P, N], bf16, name="ar_sb")
            bias_sb = cpool.tile([P, MT], fp32, name="bias_sb")
            # Chunk (g0, mt0) is split into two 1024-wide sub-chunks so the
            # pipeline starts as soon as the first half of the g0 weights
            # lands; mt0 therefore owns NGROUPS+1 partial-sum columns.
            acc_sb = cpool.tile([P, MT * NGROUPS + 1], fp32, name="acc_sb")
            ones_sb = cpool.tile([P, NGROUPS + 1], fp32, name="ones_sb")
            y_sb = cpool.tile([P, MT], fp32, name="y_sb")
            warm_sb = cpool.tile([P, 1], fp32, name="warm_sb")

            # DMA triggers cost ~0.65us each on the Sync engine, so order
            # them by need: xt (first LDWEIGHTS), wt g0 d0 (first matmul),
            # then the rest. Later groups overlap compute.
            def load_group(g):
                gs = slice(g * NGRP, (g + 1) * NGRP)
                nc.sync.dma_start(wt_sb[:, :, gs], wt[:, :, gs])
                nc.sync.dma_start(ar_sb[:, gs], ar[:, gs])

            # DMA triggers serialize at ~0.65us each on the Sync engine, so
            # the four transfers the first 1024-wide sub-chunk needs lead
            # the queue; everything else streams behind them.
            HLF = NGRP // 2
            nc.sync.dma_start(xt_sb[:], xt[:])
            nc.sync.dma_start(wt_sb[:, :, 0:HLF], wt[:, :, 0:HLF])
            nc.sync.dma_start(bias_sb[:], bias[:])
            nc.sync.dma_start(ar_sb[:, 0:HLF], ar[:, 0:HLF])
            nc.sync.dma_start(wt_sb[:, :, HLF:NGRP], wt[:, :, HLF:NGRP])
            nc.sync.dma_start(ar_sb[:, HLF:NGRP], ar[:, HLF:NGRP])

            nc.vector.memset(ones_sb[:], 1.0)
            # Dummy activation: hoists the ~2.7us exp table load into the
            # DMA head instead of stalling the first real chunk.
            nc.scalar.activation(
                warm_sb[:],
                ones_sb[:, 0:1],
                mybir.ActivationFunctionType.Exp,
                scale=1.0,
            )
            def do_chunk(g, mt, ci, n0, width, ps, po):
                """MMs + exp + weighted reduce for cols [n0, n0+width)."""
                ms = slice(mt * P, (mt + 1) * P)
                for d in range(DCH):
                    for t in range(width // NTILE):
                        o = t * NTILE
                        nc.tensor.matmul(
                            ps[:, po + o : po + o + NTILE],
                            lhsT=xt_sb[:, d, ms],
                            rhs=wt_sb[:, d, n0 + o : n0 + o + NTILE],
                            start=(d == 0),
                            stop=(d == DCH - 1),
                        )
                k = kpool.tile([P, width], bf16, name="k")
                nc.scalar.activation(
                    k[:],
                    ps[:, po : po + width],
                    mybir.ActivationFunctionType.Exp,
                    bias=bias_sb[:, mt : mt + 1],
                    scale=1.0,
                )
                prod = prpool.tile([P, width], bf16, name="prod")
                nc.vector.scalar_tensor_tensor(
                    prod[:],
                    k[:],
                    1.0,
                    ar_sb[:, n0 : n0 + width],
                    op0=mybir.AluOpType.mult,
                    op1=mybir.AluOpType.mult,
                    accum_out=acc_sb[:, ci : ci + 1],
                )

            # mt0 partials live in columns 0..NGROUPS (one extra for the
            # split first chunk); mt>0 in NGROUPS+1+(mt-1)*NGROUPS+g.
            def col(mt, g):
                if mt == 0:
                    return g + 1  # g0 sub-chunks use columns 0 and 1
                return NGROUPS + 1 + (mt - 1) * NGROUPS + g

            for g in range(NGROUPS):
                if g + 1 < NGROUPS:
                    load_group(g + 1)
                for mt in range(MT):
                    ps = ppool.tile([P, NGRP], fp32, name="ps")
                    if g == 0 and mt == 0:
                        do_chunk(0, 0, 0, 0, HLF, ps, 0)
                        do_chunk(0, 0, 1, HLF, HLF, ps, HLF)
                    else:
                        do_chunk(g, mt, col(mt, g), g * NGRP, NGRP, ps, 0)
            for mt in range(MT):
                # Reduce this m-tile's partial-sum columns into one column.
                w = NGROUPS + 1 if mt == 0 else NGROUPS
                c0 = 0 if mt == 0 else NGROUPS + 1 + (mt - 1) * NGROUPS
                scrf = spool.tile([P, 1], fp32, name="scrf")
                nc.vector.scalar_tensor_tensor(
                    scrf.broadcast_to((P, w)),
                    acc_sb[:, c0 : c0 + w],
                    1.0,
                    ones_sb[:, 0:w],
                    op0=mybir.AluOpType.mult,
                    op1=mybir.AluOpType.mult,
                    accum_out=y_sb[:, mt : mt + 1],
                )
            nc.sync.dma_start(y[:], y_sb[:])

    nc.compile()
    return nc


def _prep_inputs(X, X_train, alpha):
    """Host-side layout prep: transposes, casts, norm terms. O((M+N)*D) work."""
    X = np.asarray(X, dtype=np.float32)
    X_train = np.asarray(X_train, dtype=np.float32)
    alpha = np.asarray(alpha, dtype=np.float32).reshape(-1)

    sq1 = np.sum(X.astype(np.float64) ** 2, axis=1)        # [M]
    sq2 = np.sum(X_train.astype(np.float64) ** 2, axis=1)  # [N]

    # alpha' = alpha * exp(-||xtrain||^2/2); fp64 -> fp32 cast underflows to 0
    # exactly where the reference's fp32 exp does (up to the bf16 flush, which
    # only drops terms < 1e-40).
    ar_row = (alpha.astype(np.float64) * np.exp(-sq2 / 2.0)).astype(np.float32)
    ar_np = np.ascontiguousarray(
        np.broadcast_to(ar_row.astype(BF16)[None, :], (P, N))
    )

    # [P, DCH, M]: partition-major layout so each tensor loads in one DMA.
    xt_full = np.ascontiguousarray(
        X.T.astype(BF16).reshape(DCH, P, M).transpose(1, 0, 2)
    )
    wt_np = np.ascontiguousarray(
        X_train.T.astype(BF16).reshape(DCH, P, N).transpose(1, 0, 2)
    )

    bias_full = (-sq1 / 2.0).astype(np.float32)  # [M]

    in_maps = []
    for c in range(NCORES):
        mslice = slice(c * MC, (c + 1) * MC)
        xt_c = np.ascontiguousarray(xt_full[:, :, mslice])  # [P, DCH, MC]
        # bias[p, mt] = -sq1[c*MC + mt*P + p]/2
        bias_c = np.ascontiguousarray(bias_full[mslice].reshape(MT, P).T)
        in_maps.append({"xt": xt_c, "wt": wt_np, "ar": ar_np, "bias": bias_c})
    return in_maps


LAST_RES = None


def kernel(X, X_train, alpha):
    from concourse import bass_utils

    nc = _build()
    in_maps = _prep_inputs(X, X_train, alpha)
    res = bass_utils.run_bass_kernel_spmd(
        nc, in_maps, core_ids=list(range(NCORES))
    ).results
    global LAST_RES
    LAST_RES = res

    out = np.empty((M, 1), dtype=np.float32)
    for c in range(NCORES):
        yc = res[c]["y"]  # [P, MT]; column mt holds rows c*MC + mt*P .. +P
        out[c * MC : (c + 1) * MC, 0] = yc.T.reshape(MC)
    return out


if __name__ == "__main__":
    rng = np.random.default_rng(0)
    X = rng.standard_normal((M, D), dtype=np.float32)
    Xt = rng.standard_normal((N, D), dtype=np.float32)
    a = rng.standard_normal((N, 1), dtype=np.float32)
    out = kernel(X=X, X_train=Xt, alpha=a)
    print("out", out.shape, out.dtype, "nonzero:", np.count_nonzero(out))


# revision 43
# speedup vs baseline: 1.0833x; 1.0302x over previous
"""GPR surrogate prediction kernel for Trainium2 (8 NeuronCores, Bass/Tile).

Computes pred = K_star @ alpha where K_star = exp(-||x_m - xtrain_n||^2 / 2).

Math: exp(-(sq1[m] + sq2[n] - 2 x.y)/2) * alpha[n]
    = exp(x.y - sq1[m]/2) * (alpha[n] * exp(-sq2[n]/2))
so per core (M sharded 8 ways, layout [m=128 partitions, n free]):
  - TensorE:  dot[m, n] = X_c @ X_train.T            (bf16, fp32 PSUM)
  - ScalarE:  K[m, n] = exp(dot + bias[m]),  bias[m] = -sq1[m]/2  (per-partition bias)
  - VectorE:  pred[m] += sum_n K[m, n] * ar[n],  ar[n] = alpha[n]*exp(-sq2[n]/2)
              (scalar_tensor_tensor with dense bf16 out so the DVE 2x perf
              mode engages; accum_out gives the per-chunk column sum.)
The per-element K*ar product underflows fp32 exactly where the reference's
single exp underflows, so results match the reference bit-for-bit in the
all-underflow regime and to normal rounding otherwise.

Pipeline structure: n-groups outer, m-tiles inner, so compute starts after
the first 1 MB weight chunk lands and later group DMAs overlap compute.
"""

import functools

import ml_dtypes
import numpy as np

M, N, D = 4096, 8192, 256
NCORES = 8
P = 128
MC = M // NCORES          # 512 query rows per core
MT = MC // P              # 4 m-tiles per core
NTILE = 512               # matmul free dim (one PSUM bank)
NGRP = 2048               # n per PSUM group (4 banks) = one ACT batch
NGROUPS = N // NGRP       # 4
TPG = NGRP // NTILE       # 4 n-tiles per group
DCH = D // P              # 2 contraction chunks

BF16 = ml_dtypes.bfloat16


@functools.lru_cache(maxsize=1)
def _build():
    import concourse.bacc as bacc
    import concourse.mybir as mybir
    import concourse.tile as tile

    fp32 = mybir.dt.float32
    bf16 = mybir.dt.bfloat16

    nc = bacc.Bacc(
        "TRN2",
        target_bir_lowering=False,
        debug=False,
        enable_asserts=False,
        num_devices=NCORES,
    )

    xt = nc.dram_tensor("xt", [P, DCH, MC], bf16, kind="ExternalInput").ap()
    wt = nc.dram_tensor("wt", [P, DCH, N], bf16, kind="ExternalInput").ap()
    ar = nc.dram_tensor("ar", [P, N], bf16, kind="ExternalInput").ap()
    bias = nc.dram_tensor("bias", [P, MT], fp32, kind="ExternalInput").ap()
    y = nc.dram_tensor("y", [P, MT], fp32, kind="ExternalOutput").ap()

    with tile.TileContext(nc) as tc:
        with (
            tc.tile_pool(name="const", bufs=1) as cpool,
            tc.tile_pool(name="kpool", bufs=4) as kpool,
            tc.tile_pool(name="prod", bufs=4) as prpool,
            tc.tile_pool(name="scr", bufs=2) as spool,
            tc.tile_pool(name="psum", bufs=2, space="PSUM") as ppool,
        ):
            # Resident tensors
            xt_sb = cpool.tile([P, DCH, MC], bf16, name="xt_sb")
            wt_sb = cpool.tile([P, DCH, N], bf16, name="wt_sb")
            ar_sb = cpool.tile([P, N], bf16, name="ar_sb")
            bias_sb = cpool.tile([P, MT], fp32, name="bias_sb")
            # Chunk (g0, mt0) is split into two 1024-wide sub-chunks so the
            # pipeline starts as soon as the first half of the g0 weights
            # lands; mt0 therefore owns NGROUPS+1 partial-sum columns.
            acc_sb = cpool.tile([P, MT * NGROUPS + 1], fp32, name="acc_sb")
            ones_sb = cpool.tile([P, NGROUPS + 1], fp32, name="ones_sb")
            y_sb = cpool.tile([P, MT], fp32, name="y_sb")
            warm_sb = cpool.tile([P, 1], fp32, name="warm_sb")

            # DMA triggers cost ~0.65us each on the Sync engine, so order
            # them by need: xt (first LDWEIGHTS), wt g0 d0 (first matmul),
            # then the rest. Later groups overlap compute.
            def load_group(g):
                gs = slice(g * NGRP, (g + 1) * NGRP)
                for d in range(DCH):
                    nc.sync.dma_start(wt_sb[:, d, gs], wt[:, d, gs])
                nc.sync.dma_start(ar_sb[:, gs], ar[:, gs])

            # Group 0 loads in 1024-column halves, interleaved with the
            # bias/ar slices the first sub-chunk needs, so sub-chunk 0 can
            # start after ~1.3 MB instead of the full 1.5 MB + ar.
            HLF = NGRP // 2
            nc.sync.dma_start(xt_sb[:], xt[:])
            for h in range(2):
                hs = slice(h * HLF, (h + 1) * HLF)
                for d in range(DCH):
                    nc.sync.dma_start(wt_sb[:, d, hs], wt[:, d, hs])
                if h == 0:
                    nc.sync.dma_start(bias_sb[:], bias[:])
            for h in range(2):
                hs = slice(h * HLF, (h + 1) * HLF)
                nc.sync.dma_start(ar_sb[:, hs], ar[:, hs])

            nc.vector.memset(ones_sb[:], 1.0)
            # Dummy activation: hoists the ~2.7us exp table load into the
            # DMA head instead of stalling the first real chunk.
            nc.scalar.activation(
                warm_sb[:],
                ones_sb[:, 0:1],
                mybir.ActivationFunctionType.Exp,
                scale=1.0,
            )
            def do_chunk(g, mt, ci, n0, width, ps, po):
                """MMs + exp + weighted reduce for cols [n0, n0+width)."""
                ms = slice(mt * P, (mt + 1) * P)
                for d in range(DCH):
                    for t in range(width // NTILE):
                        o = t * NTILE
                        nc.tensor.matmul(
                            ps[:, po + o : po + o + NTILE],
                            lhsT=xt_sb[:, d, ms],
                            rhs=wt_sb[:, d, n0 + o : n0 + o + NTILE],
                            start=(d == 0),
                            stop=(d == DCH - 1),
                        )
                k = kpool.tile([P, width], bf16, name="k")
                nc.scalar.activation(
                    k[:],
                    ps[:, po : po + width],
                    mybir.ActivationFunctionType.Exp,
                    bias=bias_sb[:, mt : mt + 1],
                    scale=1.0,
                )
                prod = prpool.tile([P, width], bf16, name="prod")
                nc.vector.scalar_tensor_tensor(
                    prod[:],
                    k[:],
                    1.0,
                    ar_sb[:, n0 : n0 + width],
                    op0=mybir.AluOpType.mult,
                    op1=mybir.AluOpType.mult,
                    accum_out=acc_sb[:, ci : ci + 1],
                )

            # mt0 partials live in columns 0..NGROUPS (one extra for the
            # split first chunk); mt>0 in NGROUPS+1+(mt-1)*NGROUPS+g.
            def col(mt, g):
                if mt == 0:
                    return g + 1  # g0 sub-chunks use columns 0 and 1
                return NGROUPS + 1 + (mt - 1) * NGROUPS + g

            for g in range(NGROUPS):
                if g + 1 < NGROUPS:
                    load_group(g + 1)
                for mt in range(MT):
                    ps = ppool.tile([P, NGRP], fp32, name="ps")
                    if g == 0 and mt == 0:
                        do_chunk(0, 0, 0, 0, HLF, ps, 0)
                        do_chunk(0, 0, 1, HLF, HLF, ps, HLF)
                    else:
                        do_chunk(g, mt, col(mt, g), g * NGRP, NGRP, ps, 0)
            for mt in range(MT):
                # Reduce this m-tile's partial-sum columns into one column.
                w = NGROUPS + 1 if mt == 0 else NGROUPS
                c0 = 0 if mt == 0 else NGROUPS + 1 + (mt - 1) * NGROUPS
                scrf = spool.tile([P, 1], fp32, name="scrf")
                nc.vector.scalar_tensor_tensor(
                    scrf.broadcast_to((P, w)),
                    acc_sb[:, c0 : c0 + w],
                    1.0,
                    ones_sb[:, 0:w],
                    op0=mybir.AluOpType.mult,
                    op1=mybir.AluOpType.mult,
                    accum_out=y_sb[:, mt : mt + 1],
                )
            nc.sync.dma_start(y[:], y_sb[:])

    nc.compile()
    return nc


def _prep_inputs(X, X_train, alpha):
    """Host-side layout prep: transposes, casts, norm terms. O((M+N)*D) work."""
    X = np.asarray(X, dtype=np.float32)
    X_train = np.asarray(X_train, dtype=np.float32)
    alpha = np.asarray(alpha, dtype=np.float32).reshape(-1)

    sq1 = np.sum(X.astype(np.float64) ** 2, axis=1)        # [M]
    sq2 = np.sum(X_train.astype(np.float64) ** 2, axis=1)  # [N]

    # alpha' = alpha * exp(-||xtrain||^2/2); fp64 -> fp32 cast underflows to 0
    # exactly where the reference's fp32 exp does (up to the bf16 flush, which
    # only drops terms < 1e-40).
    ar_row = (alpha.astype(np.float64) * np.exp(-sq2 / 2.0)).astype(np.float32)
    ar_np = np.ascontiguousarray(
        np.broadcast_to(ar_row.astype(BF16)[None, :], (P, N))
    )

    # [P, DCH, M]: partition-major layout so each tensor loads in one DMA.
    xt_full = np.ascontiguousarray(
        X.T.astype(BF16).reshape(DCH, P, M).transpose(1, 0, 2)
    )
    wt_np = np.ascontiguousarray(
        X_train.T.astype(BF16).reshape(DCH, P, N).transpose(1, 0, 2)
    )

    bias_full = (-sq1 / 2.0).astype(np.float32)  # [M]

    in_maps = []
    for c in range(NCORES):
        mslice = slice(c * MC, (c + 1) * MC)
        xt_c = np.ascontiguousarray(xt_full[:, :, mslice])  # [P, DCH, MC]
        # bias[p, mt] = -sq1[c*MC + mt*P + p]/2
        bias_c = np.ascontiguousarray(bias_full[mslice].reshape(MT, P).T)
        in_maps.append({"xt": xt_c, "wt": wt_np, "ar": ar_np, "bias": bias_c})
    return in_maps


LAST_RES = None


def kernel(X, X_train, alpha):
    from concourse import bass_utils

    nc = _build()
    in_maps = _prep_inputs(X, X_train, alpha)
    res = bass_utils.run_bass_kernel_spmd(
        nc, in_maps, core_ids=list(range(NCORES))
    ).results
    global LAST_RES
    LAST_RES = res

    out = np.empty((M, 1), dtype=np.float32)
    for c in range(NCORES):
        yc = res[c]["y"]  # [P, MT]; column mt holds rows c*MC + mt*P .. +P
        out[c * MC : (c + 1) * MC, 0] = yc.T.reshape(MC)
    return out


if __name__ == "__main__":
    rng = np.random.default_rng(0)
    X = rng.standard_normal((M, D), dtype=np.float32)
    Xt = rng.standard_normal((N, D), dtype=np.float32)
    a = rng.standard_normal((N, 1), dtype=np.float32)
    out = kernel(X=X, X_train=Xt, alpha=a)
    print("out", out.shape, out.dtype, "nonzero:", np.count_nonzero(out))
